# revision 39
# baseline (speedup 1.0000x reference)
"""ContinuousWaveletTransform (Morlet bank, 32 scales) on 8 TRN2 cores.

Structure exploited: the reference wavelet is w[k] = exp(-0.5 k^2) *
exp(i 2pi k / 6) (bandwidth=1), so the envelope dies within a few samples
(tap 5 is 3.7e-6, tap 7 is 2.3e-11).  EVERY scale uses the SAME few
significant taps; the scale only sets a per-channel delay wl_c in
{64,194,...,2014,2048}.  Therefore

    out[b, c, n] = y[b, u(c), n + 2048 - wl_c],
    y[b, u, m]   = sum_{k<T} w_u[k] * sigp[b, m + k],   m in [0, 6080)

with sigp = [zeros(2048), signal], u in {re, im}, T = 5 taps.  The device
computes only y (one tiny T-tap complex conv, batch-packed into a
K=20 x M=8 matmul); the full (4, 32, 4096) output is a pure shift-gather
of y done during the host-side unshard (the same class of host indexing
work as the baseline's host im2col + re/im interleave).

Sharding: sequence-parallel over the 6080 y columns; core r computes
y[:, :, 760r : 760(r+1)].  Per core: ~31 KB in (fp16), ~12 KB out (fp16),
2 matmuls (512 + 248 cols, K=20, M=8), pipelined: two input DMA queues ->
matmuls -> PSUM copies (vector + scalar) -> output DMA (sync + scalar).

The program is bare per-engine instructions (no nc.Block()): the walrus
NEFF epilogue provides the final all-engine rendezvous, so no explicit
exit barrier or output-DMA completion wait is needed — the ~7 us fixed
teardown (253 semaphore resets + rendezvous) covers the DMA flight.
That fixed NEFF overhead (~8 us measured: preamble + teardown) dominates
the exec-time metric; the live span is ~5 us (input DMA latency ~2.4,
compute ~1.0, copies + output issue ~1.5).
"""

import os
import numpy as np

import concourse.bacc as bacc
from concourse import mybir
from concourse.bass_utils import run_bass_kernel_spmd

# ---------------------------------------------------------------- constants
B = 4
L = 4096
N_SCALES = 32
WLMAX = 2048
NCORES = 8
T = 5                        # taps kept per wavelet (tap 5 is 3.7e-06)
M_TOT = 6080                 # y columns needed: max n + 2048 - min wl + 1
NBLK = M_TOT // NCORES       # 760 y columns per core
NBLOCK = 2                   # column-blocks packed into the PE contraction
NCOL = NBLK // NBLOCK        # 380 matmul columns per core
H0 = NCOL // 2               # first matmul/out chunk (190 cols)
H1 = NCOL - H0               # 190: balanced tail
KB = T * B                   # 20 contraction rows per block
K_ROWS = KB * NBLOCK         # 40 total contraction rows
NCH = 2 * B * NBLOCK         # 16 output channels: (block, b, re/im)

# distinct per-scale window lengths (= delays); scales >= 16 clamp to L/2
_WLS = [64, 194, 324, 454, 584, 714, 844, 974, 1104, 1234, 1364, 1494,
        1624, 1754, 1884, 2014] + [2048] * 16

# matmul dtype: "float16" (half the DMA bytes, ~1e-3 err), "float32r"
# (1 col/cyc, ~2e-4 err), "float32" (exact, 2 passes).  All stream 1
# col/cyc on the PE at this size; fp16 wins on DMA issue+transfer time.
MM_DTYPE = os.environ.get("CWT_MM_DTYPE", "float16")
# output dtype for y: fp16 halves the copy/out-DMA bytes, ~5e-4 err
OUT_DTYPE = os.environ.get("CWT_OUT_DTYPE", "float16")


def _wavelet_taps():
    t = np.arange(T, dtype=np.float32)
    env = np.exp(-0.5 * t * t).astype(np.float32)
    ph = np.float32(2.0 * np.pi * 1.0 / 6.0) * t
    wr = (env * np.cos(ph)).astype(np.float32)
    wi = (env * np.sin(ph)).astype(np.float32)
    return wr, wi


def _build_inputs_per_core(signal):
    """Per-core [K_ROWS, 16 + 380] operand: [lhsT | rhs chunk0 | rhs chunk1].

    Two 380-col blocks are packed into the contraction dim: row q = 20*blk +
    T*b + k holds sigp[b, 760 r + 380 blk + j + k]; lhsT is block-diagonal so
    out channel 8*blk + 2b + u = y[b, u, 760 r + 380 blk + j].
    """
    wr, wi = _wavelet_taps()
    lhsT = np.zeros((K_ROWS, NCH), np.float32)
    for blk in range(NBLOCK):
        for b in range(B):
            for k in range(T):
                lhsT[KB * blk + T * b + k, 8 * blk + 2 * b + 0] = wr[k]
                lhsT[KB * blk + T * b + k, 8 * blk + 2 * b + 1] = wi[k]
    sigp = np.zeros((B, WLMAX + L + T), np.float32)
    sigp[:, WLMAX:WLMAX + L] = signal
    np_dt = mybir.dt.np(getattr(mybir.dt, MM_DTYPE))
    packed = []
    for r in range(NCORES):
        rhs = np.empty((K_ROWS, NCOL), np.float32)
        for blk in range(NBLOCK):
            m0 = NBLK * r + NCOL * blk
            for b in range(B):
                for k in range(T):
                    rhs[KB * blk + T * b + k, :] = \
                        sigp[b, m0 + k: m0 + k + NCOL]
        packed.append(np.concatenate([lhsT, rhs], axis=1).astype(np_dt))
    return packed


def _build_nc():
    dt_mm = getattr(mybir.dt, MM_DTYPE)
    dt_out = getattr(mybir.dt, OUT_DTYPE)
    nc = bacc.Bacc("TRN2", target_bir_lowering=False, debug=False,
                   num_devices=NCORES)
    C0 = NCH + H0                        # end of chunk0 (lhsT + chunk 0)
    C1 = NCH + NCOL                      # end of chunk1
    rhs_d = nc.dram_tensor("rhs", [K_ROWS, C1], dt_mm, kind="ExternalInput")
    out_d = nc.dram_tensor("out", [NCH, NCOL], dt_out,
                           kind="ExternalOutput")

    with (
        nc.sbuf_tensor("rhs_sb", [K_ROWS, C1], dt_mm) as rhs_sb,
        nc.sbuf_tensor("out_sb", [NCH, NCOL], dt_out) as out_sb,
        nc.psum_tensor("acc", [NCH, 2, 512], mybir.dt.float32) as acc,
        nc.semaphore("s_a") as s_a,
        nc.semaphore("s_b") as s_b,
        nc.semaphore("s_mm") as s_mm,
        nc.semaphore("s_cp") as s_cp,
        nc.semaphore("s_out") as s_out,
    ):
        # The whole program is bare per-engine instructions in the entry bb
        # (no nc.Block()): there is nothing to branch over, and skipping the
        # Block-exit all-engine barrier lets each engine flow straight from
        # its last work item into the walrus NEFF epilogue, whose own entry
        # rendezvous provides the final synchronization.
        #
        # Input DMAs on two independent HWDGE queues.  Column-split: mm0 is
        # gated only on chunk_a (lhsT + first chunk).
        nc.sync.dma_start(
            rhs_sb[:, 0:C0], rhs_d[:, 0:C0],
            single_packet=True).then_inc(s_a, 16)
        nc.scalar.dma_start(
            rhs_sb[:, C0:C1], rhs_d[:, C0:C1],
            single_packet=True).then_inc(s_b, 16)

        lhsT_ap = rhs_sb[:, 0:NCH]
        nc.tensor.wait_ge(s_a, 16)
        nc.tensor.matmul(
            acc[:, 0, 0:H0], lhsT_ap, rhs_sb[:, NCH:C0],
            start=True, stop=True,
        ).then_inc(s_mm, 1)
        nc.tensor.wait_ge(s_b, 16)
        nc.tensor.matmul(
            acc[:, 1, 0:H1], lhsT_ap, rhs_sb[:, C0:C1],
            start=True, stop=True,
        ).then_inc(s_mm, 1)

        nc.vector.wait_ge(s_mm, 1)
        nc.vector.tensor_copy(
            out_sb[:, 0:H0], acc[:, 0, 0:H0]).then_inc(s_cp, 1)

        # No explicit wait on output-DMA completion: the walrus NEFF epilogue
        # (per-engine drains + ~6 us of semaphore resets) covers the ~1.5 us
        # DMA flight many times over, so the teardown overlaps the output
        # transfer instead of serializing behind it.
        nc.scalar.wait_ge(s_mm, 2)
        nc.scalar.copy(
            out_sb[:, H0:NCOL], acc[:, 1, 0:H1])
        nc.scalar.dma_start(
            out_d[:, H0:NCOL], out_sb[:, H0:NCOL],
            single_packet=True).then_inc(s_out, 16)

        nc.sync.wait_ge(s_cp, 1)
        nc.sync.dma_start(
            out_d[:, 0:H0], out_sb[:, 0:H0],
            single_packet=True).then_inc(s_out, 16)

    nc.compile()
    return nc


_NC_CACHE = {}


def _get_nc():
    key = (MM_DTYPE, OUT_DTYPE)
    if key not in _NC_CACHE:
        _NC_CACHE[key] = _build_nc()
    return _NC_CACHE[key]


def run(signal, trace=False, **spmd_kwargs):
    """Returns (out complex64 (4,32,4096), BassKernelResults)."""
    signal = np.asarray(signal, dtype=np.float32)
    assert signal.shape == (B, L)
    nc = _get_nc()
    packed = _build_inputs_per_core(signal)
    in_maps = [{"rhs": packed[r]} for r in range(NCORES)]
    res = run_bass_kernel_spmd(nc, in_maps, core_ids=list(range(NCORES)),
                               trace=trace, **spmd_kwargs)
    # Gather y then unshard: the full output is a shift-gather of y.
    y = np.empty((B, 2, M_TOT), np.float32)
    for r in range(NCORES):
        o = res.results[r]["out"]                     # [16, 380]
        for blk in range(NBLOCK):
            sl = slice(NBLK * r + NCOL * blk, NBLK * r + NCOL * (blk + 1))
            for b in range(B):
                y[b, 0, sl] = o[8 * blk + 2 * b + 0]
                y[b, 1, sl] = o[8 * blk + 2 * b + 1]
    idx = np.array([WLMAX - wl for wl in _WLS])       # (32,)
    m_idx = idx[:, None] + np.arange(L)[None, :]      # (32, 4096)
    out = (y[:, 0][:, m_idx] + 1j * y[:, 1][:, m_idx]).astype(np.complex64)
    return out, res


def kernel(signal):
    out, _ = run(signal, trace=False)
    return out


# revision 40
# speedup vs baseline: 1.0286x; 1.0286x over previous
"""ContinuousWaveletTransform (Morlet bank, 32 scales) on 8 TRN2 cores.

Structure exploited: the reference wavelet is w[k] = exp(-0.5 k^2) *
exp(i 2pi k / 6) (bandwidth=1), so the envelope dies within a few samples
(tap 5 is 3.7e-6, tap 7 is 2.3e-11).  EVERY scale uses the SAME few
significant taps; the scale only sets a per-channel delay wl_c in
{64,194,...,2014,2048}.  Therefore

    out[b, c, n] = y[b, u(c), n + 2048 - wl_c],
    y[b, u, m]   = sum_{k<T} w_u[k] * sigp[b, m + k],   m in [0, 6080)

with sigp = [zeros(2048), signal], u in {re, im}, T = 5 taps.  The device
computes only y (one tiny T-tap complex conv, batch-packed into a
K=20 x M=8 matmul); the full (4, 32, 4096) output is a pure shift-gather
of y done during the host-side unshard (the same class of host indexing
work as the baseline's host im2col + re/im interleave).

Sharding: sequence-parallel over the 6080 y columns; core r computes
y[:, :, 760r : 760(r+1)].  Per core: ~31 KB in (fp16), ~12 KB out (fp16),
2 matmuls (512 + 248 cols, K=20, M=8), pipelined: two input DMA queues ->
matmuls -> PSUM copies (vector + scalar) -> output DMA (sync + scalar).

The program is bare per-engine instructions (no nc.Block()): the walrus
NEFF epilogue provides the final all-engine rendezvous, so no explicit
exit barrier or output-DMA completion wait is needed — the ~7 us fixed
teardown (253 semaphore resets + rendezvous) covers the DMA flight.
That fixed NEFF overhead (~8 us measured: preamble + teardown) dominates
the exec-time metric; the live span is ~5 us (input DMA latency ~2.4,
compute ~1.0, copies + output issue ~1.5).
"""

import os
import numpy as np

import concourse.bacc as bacc
from concourse import mybir
from concourse.bass_utils import run_bass_kernel_spmd

# ---------------------------------------------------------------- constants
B = 4
L = 4096
N_SCALES = 32
WLMAX = 2048
NCORES = 8
T = 4                        # taps kept per wavelet (tap 4 is 3.4e-04)
M_TOT = 6080                 # y columns needed: max n + 2048 - min wl + 1
NBLK = M_TOT // NCORES       # 760 y columns per core
NBLOCK = 2                   # column-blocks packed into the PE contraction
NCOL = NBLK // NBLOCK        # 380 matmul columns per core
H0 = NCOL // 2               # first matmul/out chunk (190 cols)
H1 = NCOL - H0               # 190: balanced tail
KB = T * B                   # 20 contraction rows per block
K_ROWS = KB * NBLOCK         # 40 total contraction rows
NCH = 2 * B * NBLOCK         # 16 output channels: (block, b, re/im)

# distinct per-scale window lengths (= delays); scales >= 16 clamp to L/2
_WLS = [64, 194, 324, 454, 584, 714, 844, 974, 1104, 1234, 1364, 1494,
        1624, 1754, 1884, 2014] + [2048] * 16

# matmul dtype: "float16" (half the DMA bytes, ~1e-3 err), "float32r"
# (1 col/cyc, ~2e-4 err), "float32" (exact, 2 passes).  All stream 1
# col/cyc on the PE at this size; fp16 wins on DMA issue+transfer time.
MM_DTYPE = os.environ.get("CWT_MM_DTYPE", "float16")
# output dtype for y: fp16 halves the copy/out-DMA bytes, ~5e-4 err
OUT_DTYPE = os.environ.get("CWT_OUT_DTYPE", "float16")


def _wavelet_taps():
    t = np.arange(T, dtype=np.float32)
    env = np.exp(-0.5 * t * t).astype(np.float32)
    ph = np.float32(2.0 * np.pi * 1.0 / 6.0) * t
    wr = (env * np.cos(ph)).astype(np.float32)
    wi = (env * np.sin(ph)).astype(np.float32)
    return wr, wi


def _build_inputs_per_core(signal):
    """Per-core [K_ROWS, 16 + 380] operand: [lhsT | rhs chunk0 | rhs chunk1].

    Two 380-col blocks are packed into the contraction dim: row q = 20*blk +
    T*b + k holds sigp[b, 760 r + 380 blk + j + k]; lhsT is block-diagonal so
    out channel 8*blk + 2b + u = y[b, u, 760 r + 380 blk + j].
    """
    wr, wi = _wavelet_taps()
    lhsT = np.zeros((K_ROWS, NCH), np.float32)
    for blk in range(NBLOCK):
        for b in range(B):
            for k in range(T):
                lhsT[KB * blk + T * b + k, 8 * blk + 2 * b + 0] = wr[k]
                lhsT[KB * blk + T * b + k, 8 * blk + 2 * b + 1] = wi[k]
    sigp = np.zeros((B, WLMAX + L + T), np.float32)
    sigp[:, WLMAX:WLMAX + L] = signal
    np_dt = mybir.dt.np(getattr(mybir.dt, MM_DTYPE))
    packed = []
    for r in range(NCORES):
        rhs = np.empty((K_ROWS, NCOL), np.float32)
        for blk in range(NBLOCK):
            m0 = NBLK * r + NCOL * blk
            for b in range(B):
                for k in range(T):
                    rhs[KB * blk + T * b + k, :] = \
                        sigp[b, m0 + k: m0 + k + NCOL]
        packed.append(np.concatenate([lhsT, rhs], axis=1).astype(np_dt))
    return packed


def _build_nc():
    dt_mm = getattr(mybir.dt, MM_DTYPE)
    dt_out = getattr(mybir.dt, OUT_DTYPE)
    nc = bacc.Bacc("TRN2", target_bir_lowering=False, debug=False,
                   num_devices=NCORES)
    C0 = NCH + H0                        # end of chunk0 (lhsT + chunk 0)
    C1 = NCH + NCOL                      # end of chunk1
    rhs_d = nc.dram_tensor("rhs", [K_ROWS, C1], dt_mm, kind="ExternalInput")
    out_d = nc.dram_tensor("out", [NCH, NCOL], dt_out,
                           kind="ExternalOutput")

    with (
        nc.sbuf_tensor("rhs_sb", [K_ROWS, C1], dt_mm) as rhs_sb,
        nc.sbuf_tensor("out_sb", [NCH, NCOL], dt_out) as out_sb,
        nc.psum_tensor("acc", [NCH, 2, 512], mybir.dt.float32) as acc,
        nc.semaphore("s_a") as s_a,
        nc.semaphore("s_b") as s_b,
        nc.semaphore("s_mm") as s_mm,
        nc.semaphore("s_cp") as s_cp,
        nc.semaphore("s_out") as s_out,
    ):
        # The whole program is bare per-engine instructions in the entry bb
        # (no nc.Block()): there is nothing to branch over, and skipping the
        # Block-exit all-engine barrier lets each engine flow straight from
        # its last work item into the walrus NEFF epilogue, whose own entry
        # rendezvous provides the final synchronization.
        #
        # Input DMAs on two independent HWDGE queues.  Column-split: mm0 is
        # gated only on chunk_a (lhsT + first chunk).
        nc.sync.dma_start(
            rhs_sb[:, 0:C0], rhs_d[:, 0:C0],
            single_packet=True).then_inc(s_a, 16)
        nc.scalar.dma_start(
            rhs_sb[:, C0:C1], rhs_d[:, C0:C1],
            single_packet=True).then_inc(s_b, 16)

        lhsT_ap = rhs_sb[:, 0:NCH]
        nc.tensor.wait_ge(s_a, 16)
        nc.tensor.matmul(
            acc[:, 0, 0:H0], lhsT_ap, rhs_sb[:, NCH:C0],
            start=True, stop=True,
        ).then_inc(s_mm, 1)
        nc.tensor.wait_ge(s_b, 16)
        nc.tensor.matmul(
            acc[:, 1, 0:H1], lhsT_ap, rhs_sb[:, C0:C1],
            start=True, stop=True,
        ).then_inc(s_mm, 1)

        nc.vector.wait_ge(s_mm, 1)
        nc.vector.tensor_copy(
            out_sb[:, 0:H0], acc[:, 0, 0:H0]).then_inc(s_cp, 1)

        # No explicit wait on output-DMA completion: the walrus NEFF epilogue
        # (per-engine drains + ~6 us of semaphore resets) covers the ~1.5 us
        # DMA flight many times over, so the teardown overlaps the output
        # transfer instead of serializing behind it.
        nc.scalar.wait_ge(s_mm, 2)
        nc.scalar.copy(
            out_sb[:, H0:NCOL], acc[:, 1, 0:H1])
        nc.scalar.dma_start(
            out_d[:, H0:NCOL], out_sb[:, H0:NCOL],
            single_packet=True).then_inc(s_out, 16)

        nc.sync.wait_ge(s_cp, 1)
        nc.sync.dma_start(
            out_d[:, 0:H0], out_sb[:, 0:H0],
            single_packet=True).then_inc(s_out, 16)

    nc.compile()
    return nc


_NC_CACHE = {}


def _get_nc():
    key = (MM_DTYPE, OUT_DTYPE)
    if key not in _NC_CACHE:
        _NC_CACHE[key] = _build_nc()
    return _NC_CACHE[key]


def run(signal, trace=False, **spmd_kwargs):
    """Returns (out complex64 (4,32,4096), BassKernelResults)."""
    signal = np.asarray(signal, dtype=np.float32)
    assert signal.shape == (B, L)
    nc = _get_nc()
    packed = _build_inputs_per_core(signal)
    in_maps = [{"rhs": packed[r]} for r in range(NCORES)]
    res = run_bass_kernel_spmd(nc, in_maps, core_ids=list(range(NCORES)),
                               trace=trace, **spmd_kwargs)
    # Gather y then unshard: the full output is a shift-gather of y.
    y = np.empty((B, 2, M_TOT), np.float32)
    for r in range(NCORES):
        o = res.results[r]["out"]                     # [16, 380]
        for blk in range(NBLOCK):
            sl = slice(NBLK * r + NCOL * blk, NBLK * r + NCOL * (blk + 1))
            for b in range(B):
                y[b, 0, sl] = o[8 * blk + 2 * b + 0]
                y[b, 1, sl] = o[8 * blk + 2 * b + 1]
    idx = np.array([WLMAX - wl for wl in _WLS])       # (32,)
    m_idx = idx[:, None] + np.arange(L)[None, :]      # (32, 4096)
    out = (y[:, 0][:, m_idx] + 1j * y[:, 1][:, m_idx]).astype(np.complex64)
    return out, res


def kernel(signal):
    out, _ = run(signal, trace=False)
    return out
